# revision 11
# baseline (speedup 1.0000x reference)
"""Causal multi-head attention on 8 Trainium2 NeuronCores.

Problem: B=4, T=2048, D=2048, H=16 heads, HD=128.
  q = x@Wq.T, k = x@Wk.T, v = x@Wv.T  (per-head causal softmax(q k^T/sqrt(hd)) v)
  out = ctx@Wo.T + b_out

Sharding: batch(4) x head-group(2) grid over 8 cores. Core c handles batch
b=c//2 and heads [8g, 8g+8) with g=c%2. Wq/Wk/Wv split column-wise (head
slices), Wo row-wise; each core emits a partial [T, D] output (bf16) and the
host sums pairs in f32 and adds b_out.

Precision: fp8(e4m3)+DoubleRow matmuls (2x PE rate) for the Q/K/V
projections of tokens 512.., bf16 for tokens 0..511 (softmax averaging
suppresses fp8 noise by ~1/sqrt(n_keys); early tokens lack that averaging,
so they get bf16). fp8 operands are pre-scaled by 32 on the host to clear
the e4m3 subnormal floor; the 1/32 is undone in the PSUM->SBUF copy, so
q/k/v land in SBUF as true-scale bf16 and downstream is uniform. Attention
and the output projection are bf16 with f32 PSUM.

Structure (all staging in SBUF, no DRAM round trips):
  P1 (projections, per head h): qT_h/kT_h in [hd, T] layout, V in [T, hd]
     pair tiles, contraction D on partitions (host provides x.T).
  P2 (attention, head h emitted with P1 of head h+2 so the tensor engine
     stays dense while ACT runs exp): scores transposed sT[k,q] = K_chunk @
     QT, p = exp(sT - 10) on ACT (fixed offset; scores are O(1) so no max
     pass), causal mask via affine_select fill-0 after exp, softmax sum via
     a bf16 pairwise add-tree on DVE + one ones-matmul per (h, q-chunk),
     PV accumulated over k-tiles, deferred normalization
     ctx *= partition_broadcast(1/l).
  P3: out[t, :] = sum_h ctx_h[:, t].T @ Wo_h accumulated in PSUM, streamed
     in two Wo column halves.

The 1/sqrt(HD) score scale is folded into Wq on the host.
"""

import math
import numpy as np
import ml_dtypes
from contextlib import ExitStack

import concourse.bacc as bacc
import concourse.mybir as mybir
import concourse.tile as tile
from concourse.bass_utils import run_bass_kernel_spmd

B, T, D = 4, 2048, 2048
H, HD = 16, 128
P = 128
N_CORES = 8
HEADS_PER_CORE = H // 2          # 8 heads per core (head-group split)
DL = HEADS_PER_CORE * HD         # 1024 local projection dims per core
KC = D // P                      # 16 contraction chunks
DC = KC // 2                     # 8 DoubleRow contraction chunks of 256
TT = T // P                      # 16 token tiles of 128
QC = T // 512                    # 4 query chunks of 512
PRE = 512                        # tokens computed in bf16 (rest fp8)
PTT = PRE // P                   # 4 prefix token tiles
EXP_BIAS = -10.0                 # exp(s + EXP_BIAS); cancels in normalization
EXP_BIAS8 = -5.0                 # fp8-p path: keeps p inside e4m3 range
SW = 32.0                        # host pre-scale for fp8 operands

F32 = mybir.dt.float32
BF16 = mybir.dt.bfloat16
FP8 = mybir.dt.float8e4
BF16_NP = ml_dtypes.bfloat16
FP8_NP = ml_dtypes.float8_e4m3
DR = mybir.MatmulPerfMode.DoubleRow

LAG = 2                          # P2 head h emitted alongside P1 head h+LAG
SLEAD = 3                        # score-matmul lead over l/c matmuls in P2

_CACHE = {}


def _build(repeat=1):
    nc = bacc.Bacc(None, target_bir_lowering=False)

    xT8 = nc.dram_tensor("xT8", [D, T], FP8, kind="ExternalInput")
    xTp = nc.dram_tensor("xTp", [D, PRE], BF16, kind="ExternalInput")
    wq8 = nc.dram_tensor("wq8", [HEADS_PER_CORE, P, KC, P], FP8, kind="ExternalInput")
    wk8 = nc.dram_tensor("wk8", [HEADS_PER_CORE, P, KC, P], FP8, kind="ExternalInput")
    wv8 = nc.dram_tensor("wv8", [HEADS_PER_CORE // 2, P, KC, 256], FP8, kind="ExternalInput")
    wqp = nc.dram_tensor("wqp", [HEADS_PER_CORE, P, KC, P], BF16, kind="ExternalInput")
    wkp = nc.dram_tensor("wkp", [HEADS_PER_CORE, P, KC, P], BF16, kind="ExternalInput")
    wvp = nc.dram_tensor("wvp", [HEADS_PER_CORE // 2, P, KC, 256], BF16, kind="ExternalInput")
    woT = nc.dram_tensor("woT", [DL, D], BF16, kind="ExternalInput")
    out = nc.dram_tensor("out", [T, D], BF16, kind="ExternalOutput")

    with tile.TileContext(nc) as tc:
        with ExitStack() as octx:
            xp = octx.enter_context(tc.tile_pool(name="xp", bufs=1))
            xpp = octx.enter_context(tc.tile_pool(name="xpp", bufs=1))
            qkp = octx.enter_context(tc.tile_pool(name="qkp", bufs=3))
            vp = octx.enter_context(tc.tile_pool(name="vp", bufs=3))
            ctxp = octx.enter_context(tc.tile_pool(name="ctxp", bufs=8))
            wqkp = octx.enter_context(tc.tile_pool(name="wqkp", bufs=2))
            wvpp = octx.enter_context(tc.tile_pool(name="wvpp", bufs=2))
            wop = octx.enter_context(tc.tile_pool(name="wop", bufs=9))
            pp = octx.enter_context(tc.tile_pool(name="pp", bufs=7))
            trp = octx.enter_context(tc.tile_pool(name="trp", bufs=5))
            otp = octx.enter_context(tc.tile_pool(name="otp", bufs=2))
            nrm = octx.enter_context(tc.tile_pool(name="nrm", bufs=2))
            msc = octx.enter_context(tc.tile_pool(name="msc", bufs=1))
            ps_a = octx.enter_context(tc.tile_pool(name="ps_a", bufs=2, space="PSUM"))
            ps_s = octx.enter_context(tc.tile_pool(name="ps_s", bufs=3, space="PSUM"))
            ps_c = octx.enter_context(tc.tile_pool(name="ps_c", bufs=2, space="PSUM"))
            ps_l = octx.enter_context(tc.tile_pool(name="ps_l", bufs=1, space="PSUM"))

            ones = msc.tile([P, P], BF16, tag="ones")
            nc.vector.memset(ones[:], 1.0)
            ones8 = msc.tile([P, 2, P], FP8, tag="ones8")
            nc.vector.memset(ones8[:], 1.0)
            ebias = msc.tile([P, 1], F32, tag="ebias")
            nc.vector.memset(ebias[:], EXP_BIAS)
            ebias8 = msc.tile([P, 1], F32, tag="ebias8")
            nc.vector.memset(ebias8[:], EXP_BIAS8)

            xT8_r = xT8.rearrange("(kc p) t -> p kc t", p=P)
            xTp_r = xTp.rearrange("(kc p) t -> p kc t", p=P)

            for _rep in range(repeat):
                x8_t = xp.tile([P, KC, T], FP8, tag="x8")
                for kc in range(KC):
                    nc.sync.dma_start(x8_t[:, kc, :], xT8_r[:, kc, :])
                xp_t = xpp.tile([P, KC, PRE], BF16, tag="xpre")
                nc.sync.dma_start(xp_t[:], xTp_r[:])

                q_tiles, k_tiles, v_tiles, ctx_tiles = {}, {}, {}, {}

                def emit_p1_head(h):
                    for w8, wp_, store in ((wq8, wqp, q_tiles), (wk8, wkp, k_tiles)):
                        wm8 = wqkp.tile([P, KC, P], FP8, tag="wqk8")
                        nc.sync.dma_start(wm8[:], w8[h])
                        wmp = wqkp.tile([P, KC, P], BF16, tag="wqkp")
                        nc.sync.dma_start(wmp[:], wp_[h])
                        dst = qkp.tile([P, T], BF16,
                                       tag="q" if store is q_tiles else "k")
                        store[h] = dst
                        # prefix tokens 0..511 in bf16
                        ps = ps_a.tile([P, 512], F32, tag="psa")
                        for kc in range(KC):
                            nc.tensor.matmul(
                                ps[:], wmp[:, kc, :], xp_t[:, kc, :],
                                start=(kc == 0), stop=(kc == KC - 1))
                        nc.vector.tensor_copy(dst[:, 0:512], ps[:])
                        # tokens 512.. in fp8 DoubleRow
                        for t4 in range(1, QC):
                            ps = ps_a.tile([P, 512], F32, tag="psa")
                            for dc in range(DC):
                                nc.tensor.matmul(
                                    ps[:],
                                    wm8[:, 2 * dc:2 * dc + 2, :],
                                    x8_t[:, 2 * dc:2 * dc + 2,
                                         t4 * 512:(t4 + 1) * 512],
                                    start=(dc == 0), stop=(dc == DC - 1),
                                    perf_mode=DR)
                            nc.vector.tensor_copy(
                                dst[:, t4 * 512:(t4 + 1) * 512], ps[:])
                    if h % 2 == 1:
                        j = h // 2
                        wvm8 = wvpp.tile([P, KC, 256], FP8, tag="wv8")
                        nc.sync.dma_start(wvm8[:], wv8[j])
                        wvmp = wvpp.tile([P, KC, 256], BF16, tag="wvp")
                        nc.sync.dma_start(wvmp[:], wvp[j])
                        vt = vp.tile([P, PTT, 256], BF16, tag="v")
                        v8t = vp.tile([P, TT, 256], FP8, tag="v8")
                        v_tiles[j] = (vt, v8t)
                        for tt in range(PTT):
                            ps = ps_a.tile([P, 256], F32, tag="psa")
                            for kc in range(KC):
                                nc.tensor.matmul(
                                    ps[:],
                                    xp_t[:, kc, tt * P:(tt + 1) * P],
                                    wvmp[:, kc, :],
                                    start=(kc == 0), stop=(kc == KC - 1))
                            nc.vector.tensor_copy(vt[:, tt, :], ps[:])
                            nc.vector.tensor_copy(v8t[:, tt, :], ps[:])
                        for tt in range(PTT, TT):
                            ps = ps_a.tile([P, 256], F32, tag="psa")
                            for dc in range(DC):
                                nc.tensor.matmul(
                                    ps[:],
                                    x8_t[:, 2 * dc:2 * dc + 2,
                                         tt * P:(tt + 1) * P],
                                    wvm8[:, 2 * dc:2 * dc + 2, :],
                                    start=(dc == 0), stop=(dc == DC - 1),
                                    perf_mode=DR)
                            nc.vector.tensor_copy(v8t[:, tt, :], ps[:])

                def emit_p2_head(h):
                    qh = q_tiles.pop(h)
                    kh = k_tiles.pop(h)
                    vt, v8t = v_tiles[h // 2]
                    hs = (h % 2) * P
                    ctx_h = ctxp.tile([P, T], BF16, tag="ctx")
                    ctx_tiles[h] = ctx_h

                    for qc2 in range(2 * QC):
                        npair = qc2 + 1          # pairs of 128-wide k tiles
                        fp8p = qc2 >= 2          # rows >= 512: fp8 p
                        qs = slice(qc2 * 256, (qc2 + 1) * 256)
                        l_ps = ps_l.tile([P, 256], F32, tag="l")
                        c_ps = ps_c.tile([P, 256], F32, tag="c")

                        pTs = {}

                        def emit_s(pi):
                            s_ps = ps_s.tile([P, 2, 256], F32, tag="s")
                            for j2 in range(2):
                                ki = 2 * pi + j2
                                nc.tensor.matmul(
                                    s_ps[:, j2, :],
                                    kh[:, ki * P:(ki + 1) * P],
                                    qh[:, qs],
                                    start=True, stop=True)
                            pT = pp.tile([P, 2, 256], FP8 if fp8p else BF16,
                                         tag="pT8" if fp8p else "pT")
                            nc.scalar.activation(
                                pT[:], s_ps[:],
                                mybir.ActivationFunctionType.Exp,
                                bias=(ebias8 if fp8p else ebias)[:],
                                scale=1.0 / (SW * SW))
                            if pi == qc2:
                                # the last pair holds the diagonal tiles
                                for j2 in range(2):
                                    nc.gpsimd.affine_select(
                                        out=pT[:, j2, :], in_=pT[:, j2, :],
                                        compare_op=mybir.AluOpType.is_ge,
                                        fill=0.0, base=-P * j2,
                                        channel_multiplier=-1,
                                        pattern=[[1, 256]])
                            pTs[pi] = pT

                        for pi in range(min(SLEAD, npair)):
                            emit_s(pi)
                        for pi in range(npair):
                            if pi + SLEAD < npair:
                                emit_s(pi + SLEAD)
                            pT = pTs.pop(pi)
                            if fp8p:
                                nc.tensor.matmul(
                                    l_ps[:], ones8[:], pT[:],
                                    start=(pi == 0), stop=(pi == npair - 1),
                                    perf_mode=DR)
                                nc.tensor.matmul(
                                    c_ps[:],
                                    v8t[:, 2 * pi:2 * pi + 2, hs:hs + P],
                                    pT[:],
                                    start=(pi == 0), stop=(pi == npair - 1),
                                    perf_mode=DR)
                            else:
                                for j2 in range(2):
                                    ki = 2 * pi + j2
                                    st = (pi == 0 and j2 == 0)
                                    sp = (pi == npair - 1 and j2 == 1)
                                    nc.tensor.matmul(
                                        l_ps[:], ones[:], pT[:, j2, :],
                                        start=st, stop=sp)
                                    nc.tensor.matmul(
                                        c_ps[:], vt[:, ki, hs:hs + P],
                                        pT[:, j2, :],
                                        start=st, stop=sp)
                        rl = nrm.tile([P, 256], F32, tag="rl")
                        nc.vector.reciprocal(rl[:], l_ps[:])
                        nc.vector.tensor_mul(ctx_h[:, qs], c_ps[:], rl[:])
                    if h % 2 == 1:
                        v_tiles.pop(h // 2)

                for h in range(HEADS_PER_CORE + LAG):
                    if h < HEADS_PER_CORE:
                        emit_p1_head(h)
                    if h >= LAG:
                        emit_p2_head(h - LAG)

                # ---------------- P3: output projection ----------------
                for ocH in range(2):
                    wo_t = []
                    for h in range(HEADS_PER_CORE):
                        wt = wop.tile([P, 1024], BF16, tag="wo")
                        nc.sync.dma_start(
                            wt[:], woT[h * P:(h + 1) * P,
                                       ocH * 1024:(ocH + 1) * 1024])
                        wo_t.append(wt)
                    for tt in range(TT):
                        ot = otp.tile([P, 1024], BF16, tag="ot")
                        for oc2 in range(2):
                            ps = ps_a.tile([P, 512], F32, tag="psa")
                            for h in range(HEADS_PER_CORE):
                                nc.tensor.matmul(
                                    ps[:],
                                    ctx_tiles[h][:, tt * P:(tt + 1) * P],
                                    wo_t[h][:, oc2 * 512:(oc2 + 1) * 512],
                                    start=(h == 0),
                                    stop=(h == HEADS_PER_CORE - 1))
                            nc.scalar.copy(
                                ot[:, oc2 * 512:(oc2 + 1) * 512], ps[:])
                        nc.sync.dma_start(
                            out[tt * P:(tt + 1) * P,
                                ocH * 1024:(ocH + 1) * 1024], ot[:])

    nc.compile()
    return nc


def _get_nc(repeat=1):
    if repeat not in _CACHE:
        _CACHE[repeat] = _build(repeat)
    return _CACHE[repeat]


def make_in_maps(inputs):
    x = np.asarray(inputs["x"], dtype=np.float32)
    Wq = np.asarray(inputs["Wq"], dtype=np.float32)
    Wk = np.asarray(inputs["Wk"], dtype=np.float32)
    Wv = np.asarray(inputs["Wv"], dtype=np.float32)
    Wo = np.asarray(inputs["Wo"], dtype=np.float32)

    scale = 1.0 / math.sqrt(HD)

    def heads4(A, grp, dt):
        # A: [DL, D] -> [n_grp, P_partition, KC, grp] with
        # out[j, p, kc, m] = A[j*grp + m, kc*128 + p]
        n = DL // grp
        return np.ascontiguousarray(
            A.reshape(n, grp, KC, P).transpose(0, 3, 2, 1).astype(dt))

    in_maps = []
    for c in range(N_CORES):
        b, g = divmod(c, 2)
        hs = slice(g * DL, (g + 1) * DL)
        xTb = x[b].T
        in_maps.append({
            "xT8": np.ascontiguousarray(xTb.astype(FP8_NP)),
            "xTp": np.ascontiguousarray(xTb[:, :PRE].astype(BF16_NP)),
            "wq8": heads4(Wq[hs, :] * (scale * SW), P, FP8_NP),
            "wk8": heads4(Wk[hs, :] * SW, P, FP8_NP),
            "wv8": heads4(Wv[hs, :] * SW, 256, FP8_NP),
            "wqp": heads4(Wq[hs, :] * (scale * SW), P, BF16_NP),
            "wkp": heads4(Wk[hs, :] * SW, P, BF16_NP),
            "wvp": heads4(Wv[hs, :] * SW, 256, BF16_NP),
            "woT": np.ascontiguousarray((Wo[:, hs].T / SW).astype(BF16_NP)),
        })
    return in_maps


def run(inputs, trace=False, repeat=1):
    in_maps = make_in_maps(inputs)
    b_out = np.asarray(inputs["b_out"], dtype=np.float32)

    nc = _get_nc(repeat)
    res = run_bass_kernel_spmd(nc, in_maps, core_ids=list(range(N_CORES)),
                               trace=trace)
    outp = np.empty((B, T, D), dtype=np.float32)
    for b in range(B):
        outp[b] = (res.results[2 * b]["out"].astype(np.float32)
                   + res.results[2 * b + 1]["out"].astype(np.float32))
    outp += b_out[None, None, :]
    return outp, res


def kernel(**inputs) -> np.ndarray:
    outp, _ = run(inputs, trace=False)
    return outp


# revision 14
# speedup vs baseline: 1.3769x; 1.3769x over previous
"""Causal multi-head attention on 8 Trainium2 NeuronCores.

Problem: B=4, T=2048, D=2048, H=16 heads, HD=128.
  q = x@Wq.T, k = x@Wk.T, v = x@Wv.T  (per-head causal softmax(q k^T/sqrt(hd)) v)
  out = ctx@Wo.T + b_out

Sharding: batch(4) x head-group(2) grid over 8 cores. Core c handles batch
b=c//2 and heads [8g, 8g+8) with g=c%2. Wq/Wk/Wv split column-wise (head
slices), Wo row-wise; each core emits a partial [T, D] output (bf16) and the
host sums pairs in f32 and adds b_out.

Precision: fp8(e4m3)+DoubleRow matmuls (2x PE rate) for the Q/K/V
projections of tokens 512.., bf16 for tokens 0..511 (softmax averaging
suppresses fp8 noise by ~1/sqrt(n_keys); early tokens lack that averaging,
so they get bf16). fp8 operands are pre-scaled by 32 on the host to clear
the e4m3 subnormal floor; the 1/32 is undone in the PSUM->SBUF copy, so
q/k/v land in SBUF as true-scale bf16 and downstream is uniform. Attention
and the output projection are bf16 with f32 PSUM.

Structure (all staging in SBUF, no DRAM round trips):
  P1 (projections, per head h): qT_h/kT_h in [hd, T] layout, V in [T, hd]
     pair tiles, contraction D on partitions (host provides x.T).
  P2 (attention, head h emitted with P1 of head h+2 so the tensor engine
     stays dense while ACT runs exp): scores transposed sT[k,q] = K_chunk @
     QT, p = exp(sT - 10) on ACT (fixed offset; scores are O(1) so no max
     pass), causal mask via affine_select fill-0 after exp, softmax sum via
     a bf16 pairwise add-tree on DVE + one ones-matmul per (h, q-chunk),
     PV accumulated over k-tiles, deferred normalization
     ctx *= partition_broadcast(1/l).
  P3: out[t, :] = sum_h ctx_h[:, t].T @ Wo_h accumulated in PSUM, streamed
     in two Wo column halves.

The 1/sqrt(HD) score scale is folded into Wq on the host.
"""

import math
import numpy as np
import ml_dtypes
from contextlib import ExitStack

import concourse.bacc as bacc
import concourse.mybir as mybir
import concourse.tile as tile
from concourse.bass_utils import run_bass_kernel_spmd

B, T, D = 4, 2048, 2048
H, HD = 16, 128
P = 128
N_CORES = 8
HEADS_PER_CORE = H // 2          # 8 heads per core (head-group split)
DL = HEADS_PER_CORE * HD         # 1024 local projection dims per core
KC = D // P                      # 16 contraction chunks
DC = KC // 2                     # 8 DoubleRow contraction chunks of 256
TT = T // P                      # 16 token tiles of 128
QC = T // 512                    # 4 query chunks of 512
PRE = 512                        # tokens computed in bf16 (rest fp8)
PTT = PRE // P                   # 4 prefix token tiles
EXP_BIAS = -10.0                 # exp(s + EXP_BIAS); cancels in normalization
EXP_BIAS8 = -5.0                 # fp8-p path: keeps p inside e4m3 range
SW = 32.0                        # host pre-scale for fp8 operands

F32 = mybir.dt.float32
BF16 = mybir.dt.bfloat16
FP8 = mybir.dt.float8e4
BF16_NP = ml_dtypes.bfloat16
FP8_NP = ml_dtypes.float8_e4m3
DR = mybir.MatmulPerfMode.DoubleRow

LAG = 2                          # P2 head h emitted alongside P1 head h+LAG
SLEAD = 3                        # score-matmul lead over l/c matmuls in P2

_CACHE = {}


def _build(repeat=1):
    nc = bacc.Bacc(None, target_bir_lowering=False)

    xT8 = nc.dram_tensor("xT8", [D, T], FP8, kind="ExternalInput")
    xTp = nc.dram_tensor("xTp", [D, PRE], BF16, kind="ExternalInput")
    wq8 = nc.dram_tensor("wq8", [HEADS_PER_CORE, P, KC, P], FP8, kind="ExternalInput")
    wk8 = nc.dram_tensor("wk8", [HEADS_PER_CORE, P, KC, P], FP8, kind="ExternalInput")
    wv8 = nc.dram_tensor("wv8", [HEADS_PER_CORE // 2, P, KC, 256], FP8, kind="ExternalInput")
    wqp = nc.dram_tensor("wqp", [HEADS_PER_CORE, P, KC, P], BF16, kind="ExternalInput")
    wkp = nc.dram_tensor("wkp", [HEADS_PER_CORE, P, KC, P], BF16, kind="ExternalInput")
    wvp = nc.dram_tensor("wvp", [HEADS_PER_CORE // 2, P, KC, 256], BF16, kind="ExternalInput")
    woT = nc.dram_tensor("woT", [DL, D], BF16, kind="ExternalInput")
    out = nc.dram_tensor("out", [T, D], BF16, kind="ExternalOutput")

    with tile.TileContext(nc) as tc:
        with ExitStack() as octx:
            xp = octx.enter_context(tc.tile_pool(name="xp", bufs=1))
            xpp = octx.enter_context(tc.tile_pool(name="xpp", bufs=1))
            qkp = octx.enter_context(tc.tile_pool(name="qkp", bufs=3))
            vp = octx.enter_context(tc.tile_pool(name="vp", bufs=3))
            ctxp = octx.enter_context(tc.tile_pool(name="ctxp", bufs=8))
            wqkp = octx.enter_context(tc.tile_pool(name="wqkp", bufs=2))
            wvpp = octx.enter_context(tc.tile_pool(name="wvpp", bufs=2))
            wop = octx.enter_context(tc.tile_pool(name="wop", bufs=9))
            pp = octx.enter_context(tc.tile_pool(name="pp", bufs=7))
            trp = octx.enter_context(tc.tile_pool(name="trp", bufs=5))
            otp = octx.enter_context(tc.tile_pool(name="otp", bufs=2))
            nrm = octx.enter_context(tc.tile_pool(name="nrm", bufs=2))
            msc = octx.enter_context(tc.tile_pool(name="msc", bufs=1))
            ps_a = octx.enter_context(tc.tile_pool(name="ps_a", bufs=2, space="PSUM"))
            ps_s = octx.enter_context(tc.tile_pool(name="ps_s", bufs=3, space="PSUM"))
            ps_c = octx.enter_context(tc.tile_pool(name="ps_c", bufs=2, space="PSUM"))
            ps_l = octx.enter_context(tc.tile_pool(name="ps_l", bufs=1, space="PSUM"))

            ones = msc.tile([P, P], BF16, tag="ones")
            nc.vector.memset(ones[:], 1.0)
            ones8 = msc.tile([P, 2, P], FP8, tag="ones8")
            nc.vector.memset(ones8[:], 1.0)
            ebias = msc.tile([P, 1], F32, tag="ebias")
            nc.vector.memset(ebias[:], EXP_BIAS)
            ebias8 = msc.tile([P, 1], F32, tag="ebias8")
            nc.vector.memset(ebias8[:], EXP_BIAS8)

            xT8_r = xT8.rearrange("(kc p) t -> p kc t", p=P)
            xTp_r = xTp.rearrange("(kc p) t -> p kc t", p=P)

            for _rep in range(repeat):
                x8_t = xp.tile([P, KC, T], FP8, tag="x8")
                for kc in range(KC):
                    nc.sync.dma_start(x8_t[:, kc, :], xT8_r[:, kc, :])
                xp_t = xpp.tile([P, KC, PRE], BF16, tag="xpre")
                nc.sync.dma_start(xp_t[:], xTp_r[:])

                q_tiles, k_tiles, v_tiles, ctx_tiles = {}, {}, {}, {}

                def emit_p1_head(h):
                    for w8, wp_, store in ((wq8, wqp, q_tiles), (wk8, wkp, k_tiles)):
                        wm8 = wqkp.tile([P, KC, P], FP8, tag="wqk8")
                        nc.sync.dma_start(wm8[:], w8[h])
                        wmp = wqkp.tile([P, KC, P], BF16, tag="wqkp")
                        nc.sync.dma_start(wmp[:], wp_[h])
                        dst = qkp.tile([P, T], BF16,
                                       tag="q" if store is q_tiles else "k")
                        store[h] = dst
                        # prefix tokens 0..511 in bf16
                        ps = ps_a.tile([P, 512], F32, tag="psa")
                        for kc in range(KC):
                            nc.tensor.matmul(
                                ps[:], wmp[:, kc, :], xp_t[:, kc, :],
                                start=(kc == 0), stop=(kc == KC - 1))
                        nc.vector.tensor_copy(dst[:, 0:512], ps[:])
                        # tokens 512.. in fp8 DoubleRow
                        for t4 in range(1, QC):
                            ps = ps_a.tile([P, 512], F32, tag="psa")
                            for dc in range(DC):
                                nc.tensor.matmul(
                                    ps[:],
                                    wm8[:, 2 * dc:2 * dc + 2, :],
                                    x8_t[:, 2 * dc:2 * dc + 2,
                                         t4 * 512:(t4 + 1) * 512],
                                    start=(dc == 0), stop=(dc == DC - 1),
                                    perf_mode=DR)
                            nc.vector.tensor_copy(
                                dst[:, t4 * 512:(t4 + 1) * 512], ps[:])
                    if h % 2 == 1:
                        j = h // 2
                        wvm8 = wvpp.tile([P, KC, 256], FP8, tag="wv8")
                        nc.sync.dma_start(wvm8[:], wv8[j])
                        wvmp = wvpp.tile([P, KC, 256], BF16, tag="wvp")
                        nc.sync.dma_start(wvmp[:], wvp[j])
                        vt = vp.tile([P, PTT, 256], BF16, tag="v")
                        v8t = vp.tile([P, TT, 256], FP8, tag="v8")
                        v_tiles[j] = (vt, v8t)
                        for tt in range(PTT):
                            ps = ps_a.tile([P, 256], F32, tag="psa")
                            for kc in range(KC):
                                nc.tensor.matmul(
                                    ps[:],
                                    xp_t[:, kc, tt * P:(tt + 1) * P],
                                    wvmp[:, kc, :],
                                    start=(kc == 0), stop=(kc == KC - 1))
                            nc.vector.tensor_copy(vt[:, tt, :], ps[:])
                            nc.vector.tensor_copy(v8t[:, tt, :], ps[:])
                        for tt in range(PTT, TT):
                            ps = ps_a.tile([P, 256], F32, tag="psa")
                            for dc in range(DC):
                                nc.tensor.matmul(
                                    ps[:],
                                    x8_t[:, 2 * dc:2 * dc + 2,
                                         tt * P:(tt + 1) * P],
                                    wvm8[:, 2 * dc:2 * dc + 2, :],
                                    start=(dc == 0), stop=(dc == DC - 1),
                                    perf_mode=DR)
                            nc.vector.tensor_copy(v8t[:, tt, :], ps[:])

                def emit_p2_head(h):
                    qh = q_tiles.pop(h)
                    kh = k_tiles.pop(h)
                    vt, v8t = v_tiles[h // 2]
                    hs = (h % 2) * P
                    ctx_h = ctxp.tile([P, T], BF16, tag="ctx")
                    ctx_tiles[h] = ctx_h

                    for qc in range(QC):
                        nkt = 4 * qc + 4
                        fp8p = qc >= 1          # rows >= 512: fp8 p, DR PV
                        qs = slice(qc * 512, (qc + 1) * 512)
                        l_ps = ps_l.tile([P, 512], F32, tag="l")
                        c_ps = ps_c.tile([P, 512], F32, tag="c")

                        pTs = {}

                        def emit_s(ki):
                            s_ps = ps_s.tile([P, 512], F32, tag="s")
                            nc.tensor.matmul(
                                s_ps[:],
                                kh[:, ki * P:(ki + 1) * P],
                                qh[:, qs],
                                start=True, stop=True)
                            if fp8p:
                                if ki % 2 == 0:
                                    pT = pp.tile([P, 2, 1024], FP8, tag="pT8")
                                    pTs[ki // 2] = pT
                                dst = pTs[ki // 2][:, ki % 2, 0:512]
                            else:
                                pT = pp.tile([P, 512], BF16, tag="pT")
                                pTs[ki] = pT
                                dst = pT[:]
                            nc.scalar.activation(
                                dst, s_ps[:],
                                mybir.ActivationFunctionType.Exp,
                                bias=(ebias8 if fp8p else ebias)[:],
                                scale=1.0 / (SW * SW))
                            j = ki - 4 * qc
                            if j >= 0:
                                # keep iff q_rel - k_rel - 128*j >= 0
                                nc.gpsimd.affine_select(
                                    out=dst, in_=dst,
                                    compare_op=mybir.AluOpType.is_ge,
                                    fill=0.0, base=-P * j,
                                    channel_multiplier=-1,
                                    pattern=[[1, 512]])

                        if fp8p:
                            npair = nkt // 2
                            for ki in range(min(2 * SLEAD, nkt)):
                                emit_s(ki)
                            for pi in range(npair):
                                for ki in (2 * pi + 2 * SLEAD,
                                           2 * pi + 2 * SLEAD + 1):
                                    if ki < nkt:
                                        emit_s(ki)
                                pT = pTs.pop(pi)
                                nc.tensor.matmul(
                                    l_ps[:], ones8[:], pT[:, :, 0:512],
                                    start=(pi == 0), stop=(pi == npair - 1),
                                    perf_mode=DR)
                                nc.tensor.matmul(
                                    c_ps[:],
                                    v8t[:, 2 * pi:2 * pi + 2, hs:hs + P],
                                    pT[:, :, 0:512],
                                    start=(pi == 0), stop=(pi == npair - 1),
                                    perf_mode=DR)
                        else:
                            for ki in range(min(SLEAD, nkt)):
                                emit_s(ki)
                            for ki in range(nkt):
                                if ki + SLEAD < nkt:
                                    emit_s(ki + SLEAD)
                                pT = pTs.pop(ki)
                                nc.tensor.matmul(
                                    l_ps[:], ones[:], pT[:],
                                    start=(ki == 0), stop=(ki == nkt - 1))
                                nc.tensor.matmul(
                                    c_ps[:], vt[:, ki, hs:hs + P], pT[:],
                                    start=(ki == 0), stop=(ki == nkt - 1))
                        rl = nrm.tile([P, 512], F32, tag="rl")
                        nc.vector.reciprocal(rl[:], l_ps[:])
                        nc.vector.tensor_mul(ctx_h[:, qs], c_ps[:], rl[:])
                    if h % 2 == 1:
                        v_tiles.pop(h // 2)

                for h in range(HEADS_PER_CORE + LAG):
                    if h < HEADS_PER_CORE:
                        emit_p1_head(h)
                    if h >= LAG:
                        emit_p2_head(h - LAG)

                # ---------------- P3: output projection ----------------
                for ocH in range(2):
                    wo_t = []
                    for h in range(HEADS_PER_CORE):
                        wt = wop.tile([P, 1024], BF16, tag="wo")
                        nc.sync.dma_start(
                            wt[:], woT[h * P:(h + 1) * P,
                                       ocH * 1024:(ocH + 1) * 1024])
                        wo_t.append(wt)
                    for tt in range(TT):
                        ot = otp.tile([P, 1024], BF16, tag="ot")
                        for oc2 in range(2):
                            ps = ps_a.tile([P, 512], F32, tag="psa")
                            for h in range(HEADS_PER_CORE):
                                nc.tensor.matmul(
                                    ps[:],
                                    ctx_tiles[h][:, tt * P:(tt + 1) * P],
                                    wo_t[h][:, oc2 * 512:(oc2 + 1) * 512],
                                    start=(h == 0),
                                    stop=(h == HEADS_PER_CORE - 1))
                            nc.scalar.copy(
                                ot[:, oc2 * 512:(oc2 + 1) * 512], ps[:])
                        nc.sync.dma_start(
                            out[tt * P:(tt + 1) * P,
                                ocH * 1024:(ocH + 1) * 1024], ot[:])

    nc.compile()
    return nc


def _get_nc(repeat=1):
    if repeat not in _CACHE:
        _CACHE[repeat] = _build(repeat)
    return _CACHE[repeat]


def make_in_maps(inputs):
    x = np.asarray(inputs["x"], dtype=np.float32)
    Wq = np.asarray(inputs["Wq"], dtype=np.float32)
    Wk = np.asarray(inputs["Wk"], dtype=np.float32)
    Wv = np.asarray(inputs["Wv"], dtype=np.float32)
    Wo = np.asarray(inputs["Wo"], dtype=np.float32)

    scale = 1.0 / math.sqrt(HD)

    def heads4(A, grp, dt):
        # A: [DL, D] -> [n_grp, P_partition, KC, grp] with
        # out[j, p, kc, m] = A[j*grp + m, kc*128 + p]
        n = DL // grp
        return np.ascontiguousarray(
            A.reshape(n, grp, KC, P).transpose(0, 3, 2, 1).astype(dt))

    in_maps = []
    for c in range(N_CORES):
        b, g = divmod(c, 2)
        hs = slice(g * DL, (g + 1) * DL)
        xTb = x[b].T
        in_maps.append({
            "xT8": np.ascontiguousarray(xTb.astype(FP8_NP)),
            "xTp": np.ascontiguousarray(xTb[:, :PRE].astype(BF16_NP)),
            "wq8": heads4(Wq[hs, :] * (scale * SW), P, FP8_NP),
            "wk8": heads4(Wk[hs, :] * SW, P, FP8_NP),
            "wv8": heads4(Wv[hs, :] * SW, 256, FP8_NP),
            "wqp": heads4(Wq[hs, :] * (scale * SW), P, BF16_NP),
            "wkp": heads4(Wk[hs, :] * SW, P, BF16_NP),
            "wvp": heads4(Wv[hs, :] * SW, 256, BF16_NP),
            "woT": np.ascontiguousarray((Wo[:, hs].T / SW).astype(BF16_NP)),
        })
    return in_maps


def run(inputs, trace=False, repeat=1):
    in_maps = make_in_maps(inputs)
    b_out = np.asarray(inputs["b_out"], dtype=np.float32)

    nc = _get_nc(repeat)
    res = run_bass_kernel_spmd(nc, in_maps, core_ids=list(range(N_CORES)),
                               trace=trace)
    outp = np.empty((B, T, D), dtype=np.float32)
    for b in range(B):
        outp[b] = (res.results[2 * b]["out"].astype(np.float32)
                   + res.results[2 * b + 1]["out"].astype(np.float32))
    outp += b_out[None, None, :]
    return outp, res


def kernel(**inputs) -> np.ndarray:
    outp, _ = run(inputs, trace=False)
    return outp
